# revision 1
# baseline (speedup 1.0000x reference)
"""Self-contained Trainium2 Bass kernel for nn_Block_86028194939235 (sparse_attention).

nGPT-style block: gated-softmax causal attention + 8C MLP, B=2 T=1024 C=1024 H=16.

Sharding (8 cores, Megatron-SP style, hardcoded):
  - attention: heads sharded (2 heads/core); partial att_c_proj output
    ReduceScatter(add) over token rows -> each core owns 256 rows.
  - residual/norm work: sharded over the 256 local rows.
  - MLP: 8C hidden sharded column/row-wise; AllGather of (transposed) h1
    feeds the sharded c_fc; partial mlp_c_proj ReduceScatter'd again.
  - final output: each core returns its 256 rows; host concatenates.

All matmuls in bf16 (fp32 PSUM accumulation); comms in bf16; residual/norm
math in fp32.
"""
import math
import os

import numpy as np
import ml_dtypes

import concourse.bass as bass
import concourse.bacc as bacc
import concourse.mybir as mybir
import concourse.tile as tile
from concourse.bass import ts, ds
from concourse.bass_utils import run_bass_kernel_spmd
from concourse.masks import make_identity
from concourse.tile import add_dep_helper as _add_dep
from concourse._compat import with_exitstack, get_trn_type
from contextlib import ExitStack

NC_ = 8
B, T, C, H, D = 2, 1024, 1024, 16, 64
NT = B * T                 # 2048 tokens
HPC = H // NC_             # 2 heads per core
GD = HPC * D               # 128
ROWS = NT // NC_           # 256 rows per core
FH = 8 * C // NC_ // 2     # 512 u-rows (and 512 v-rows) per core
BASE_SCALE = 0.03125
SM_SCALE = math.sqrt(D)    # 8.0

bf16 = mybir.dt.bfloat16
f32 = mybir.dt.float32
nbf = ml_dtypes.bfloat16
AF = mybir.ActivationFunctionType


def _to_bf(x):
    return np.ascontiguousarray(np.asarray(x, np.float32).astype(nbf))


def _f32(x):
    return np.ascontiguousarray(np.asarray(x, np.float32))


# ---------------------------------------------------------------- host tables
def _sinusoidal_embeddings(n, d):
    pos = np.arange(n, dtype=np.float32)[:, None]
    div = np.exp(np.arange(0, d, 2, dtype=np.float32) * (-math.log(10000.0) / d))
    emb = np.zeros((n, d), dtype=np.float32)
    emb[:, 0::2] = np.sin(pos * div)
    emb[:, 1::2] = np.cos(pos * div)
    return emb


def _host_tables():
    emb = _sinusoidal_embeddings(T, D)
    R = np.zeros((D, T), dtype=np.float32)
    for i in range(D // 2):
        R[2 * i, :] = -emb[:, 32 + i]
        R[2 * i + 1, :] = emb[:, i]
    rope = np.tile(np.tile(R, (HPC, 1)), (1, B))          # (128, 2048)
    perm64 = np.arange(D).reshape(-1, 2)[:, ::-1].reshape(-1)
    perm128 = np.concatenate([perm64, perm64 + D])
    masks = np.zeros((4, 128, 512), dtype=np.float32)
    for m in range(4):
        masks[m] = (np.arange(512)[None, :] - 128 * m - np.arange(128)[:, None]) >= 0
    return rope, perm128, masks


def _core_inputs(g, inp, rope, perm128, masks):
    h = _f32(inp["h"]).reshape(NT, C)
    gd = slice(g * GD, (g + 1) * GD)
    sqk_s = _f32(inp["sqk"])[gd] * (1.0 / BASE_SCALE)
    normind = np.zeros((2, GD), np.float32)
    normind[np.arange(GD) // D, np.arange(GD)] = sqk_s
    ones2 = np.zeros((GD, 2), np.float32)
    ones2[np.arange(GD), np.arange(GD) // D] = 1.0
    stp = _f32(inp["stp"])
    thr = _f32(inp["thr_c"])
    gatesc = np.zeros((128, 2), np.float32)
    gatebi = np.zeros((128, 2), np.float32)
    for hi in range(HPC):
        hh = HPC * g + hi
        gatesc[:, hi] = SM_SCALE * stp[hh]
        gatebi[:, hi] = -stp[hh] * thr[hh]
    Wfc = _f32(inp["Wfc"])
    wfct = np.concatenate(
        [Wfc[g * FH:(g + 1) * FH], Wfc[4 * C + g * FH:4 * C + (g + 1) * FH]], 0).T
    suv = _f32(inp["suv"]) * math.sqrt(C)
    suvu = suv[g * FH:(g + 1) * FH].reshape(4, 128).T
    suvv = suv[4 * C + g * FH:4 * C + (g + 1) * FH].reshape(4, 128).T
    lr1 = np.abs(_f32(inp["attn_alpha"]) * (0.05 / BASE_SCALE))
    lr2 = np.abs(_f32(inp["mlp_alpha"]) * (0.05 / BASE_SCALE))
    lrs = np.concatenate([1.0 - lr1, lr1, 1.0 - lr2, lr2])
    return {
        "xt": _to_bf(h.T),
        "hloc": _f32(np.concatenate(
            [h[g * 128:(g + 1) * 128], h[T + g * 128:T + (g + 1) * 128]], 0)),
        "wqt": _to_bf(_f32(inp["Wq"])[gd][perm128].T),
        "wkt": _to_bf(_f32(inp["Wk"])[gd][perm128].T),
        "wvt": _to_bf(_f32(inp["Wv"])[gd].T),
        "wot": _to_bf(_f32(inp["Wo"])[:, gd].T),
        "rope": _f32(rope),
        "normind": _f32(normind),
        "ones2": _to_bf(ones2),
        "masks": _to_bf(masks),
        "gatesc": _f32(gatesc),
        "gatebi": _f32(gatebi),
        "wfct": _to_bf(wfct),
        "suvu": _f32(suvu),
        "suvv": _f32(suvv),
        "wprojt": _to_bf(_f32(inp["Wproj"])[:, g * FH:(g + 1) * FH].T),
        "lrs": np.ascontiguousarray(
            np.broadcast_to(lrs.reshape(1, 4 * C), (128, 4 * C)).astype(nbf)),
    }


_INPUT_SPECS = [
    ("xt", (C, NT), bf16),
    ("hloc", (ROWS, C), f32),
    ("wqt", (C, GD), bf16),
    ("wkt", (C, GD), bf16),
    ("wvt", (C, GD), bf16),
    ("wot", (GD, C), bf16),
    ("rope", (GD, NT), f32),
    ("normind", (2, GD), f32),
    ("ones2", (GD, 2), bf16),
    ("masks", (4, 128, 512), bf16),
    ("gatesc", (128, 2), f32),
    ("gatebi", (128, 2), f32),
    ("wfct", (C, 2 * FH), bf16),
    ("suvu", (128, 4), f32),
    ("suvv", (128, 4), f32),
    ("wprojt", (FH, C), bf16),
    ("lrs", (128, 4 * C), bf16),
]


# ---------------------------------------------------------------- device code
def _residual_update(nc, tmp, out_f32, a_sb, ra, upd_bf, lrm_ap, lrv_ap):
    """out = justnorm(lrm*justnorm-ish(a) + lrv*justnorm(upd)).

    a_sb: [128, C] f32 (A, pre-norm); ra: [128,1] f32 reciprocal-norm of a
          (pass None to treat a as already unit-norm).
    upd_bf: [128, C] bf16 (update branch, pre-norm).
    """
    sq = tmp.tile([128, C], bf16, tag="res_sq")
    ssb = tmp.tile([128, 1], f32, tag="res_ss")
    nc.scalar.activation(sq, upd_bf, AF.Square, accum_out=ssb)
    srt = tmp.tile([128, 1], f32, tag="res_srt")
    nc.scalar.activation(srt, ssb, AF.Sqrt)
    rb = tmp.tile([128, 1], f32, tag="res_rb")
    nc.vector.reciprocal(rb, srt)

    t1 = tmp.tile([128, C], f32, tag="res_t1")
    if ra is None:
        nc.vector.tensor_mul(t1, a_sb, lrm_ap)
    else:
        nc.vector.scalar_tensor_tensor(
            t1, in0=a_sb, scalar=ra, in1=lrm_ap,
            op0=mybir.AluOpType.mult, op1=mybir.AluOpType.mult)
    t2 = tmp.tile([128, C], f32, tag="res_t2")
    nc.vector.scalar_tensor_tensor(
        t2, in0=upd_bf, scalar=rb, in1=lrv_ap,
        op0=mybir.AluOpType.mult, op1=mybir.AluOpType.mult)
    nc.vector.tensor_add(t1, t1, t2)
    sq2 = tmp.tile([128, C], bf16, tag="res_sq")
    ss2 = tmp.tile([128, 1], f32, tag="res_ss")
    nc.scalar.activation(sq2, t1, AF.Square, accum_out=ss2)
    srt2 = tmp.tile([128, 1], f32, tag="res_srt")
    nc.scalar.activation(srt2, ss2, AF.Sqrt)
    rs = tmp.tile([128, 1], f32, tag="res_rb")
    nc.vector.reciprocal(rs, srt2)
    nc.vector.tensor_scalar_mul(out_f32, t1, rs)


def _rnorm_of(nc, tmp, x_sb):
    """reciprocal L2 norm over free dim: [128, C] f32 -> [128,1] f32."""
    sq = tmp.tile([128, C], bf16, tag="res_sq")
    ssb = tmp.tile([128, 1], f32, tag="res_ss")
    nc.scalar.activation(sq, x_sb, AF.Square, accum_out=ssb)
    srt = tmp.tile([128, 1], f32, tag="res_srt")
    nc.scalar.activation(srt, ssb, AF.Sqrt)
    r = tmp.tile([128, 1], f32, tag="res_ra")
    nc.vector.reciprocal(r, srt)
    return r


@with_exitstack
def _build_kernel(ctx: ExitStack, tc: tile.TileContext, io: dict, mock_cc=False,
                  sim_safe=False):
    nc = tc.nc
    RG = [[i for i in range(NC_)]]

    # internal DRAM for collectives
    hatt_part = nc.dram_tensor("hatt_part", [NT, C], bf16, kind="Internal").ap()
    hatt_rs = [nc.dram_tensor(f"hatt_rs{i}", [128, C], bf16, kind="Internal").ap()
               for i in range(2)]
    h1t_loc = nc.dram_tensor("h1t_loc", [C, ROWS], bf16, kind="Internal").ap()
    h1t_all = nc.dram_tensor("h1t_all", [NC_ * C, ROWS], bf16, kind="Internal",
                             addr_space="Shared").ap()
    hmlp_part = [nc.dram_tensor(f"hmlp_part{i}", [NT, 512], bf16,
                                kind="Internal").ap() for i in range(2)]
    hmlp_rs = [nc.dram_tensor(f"hmlp_rs{i}", [ROWS, 512], bf16,
                              kind="Internal").ap() for i in range(2)]

    const = ctx.enter_context(tc.tile_pool(name="const", bufs=1))
    tmp = ctx.enter_context(tc.tile_pool(name="tmp", bufs=2))
    ps = ctx.enter_context(tc.tile_pool(name="ps", bufs=2, space="PSUM"))

    # ---- load constants / weights to SBUF
    wq_sb = const.tile([128, 8, GD], bf16, tag="wq")
    nc.sync.dma_start(wq_sb, io["wqt"].rearrange("(cc p) m -> p cc m", p=128))
    xt_sb = const.tile([128, 8, NT], bf16, tag="xmat")
    xt_view = io["xt"].rearrange("(cc p) t -> p cc t", p=128)
    for cc in range(8):
        nc.sync.dma_start(xt_sb[:, cc], xt_view[:, cc])
    wk_sb = const.tile([128, 8, GD], bf16, tag="wk")
    nc.sync.dma_start(wk_sb, io["wkt"].rearrange("(cc p) m -> p cc m", p=128))
    wv_sb = const.tile([128, 8, GD], bf16, tag="wv")
    nc.sync.dma_start(wv_sb, io["wvt"].rearrange("(cc p) m -> p cc m", p=128))
    wo_sb = const.tile([128, C], bf16, tag="wo")
    nc.sync.dma_start(wo_sb, io["wot"])
    rope_sb = const.tile([128, NT], f32, tag="rope")
    nc.sync.dma_start(rope_sb, io["rope"])
    normind_sb = const.tile([2, GD], f32, tag="normind")
    nc.sync.dma_start(normind_sb, io["normind"])
    ones2_sb = const.tile([128, 2], bf16, tag="ones2")
    nc.sync.dma_start(ones2_sb, io["ones2"])
    masks_sb = const.tile([128, 4, 512], bf16, tag="masks")
    nc.sync.dma_start(masks_sb, io["masks"].rearrange("m p q -> p m q"))
    gatesc_sb = const.tile([128, 2], f32, tag="gatesc")
    nc.sync.dma_start(gatesc_sb, io["gatesc"])
    gatebi_sb = const.tile([128, 2], f32, tag="gatebi")
    nc.sync.dma_start(gatebi_sb, io["gatebi"])
    wfc_sb = const.tile([128, 8, 2 * FH], bf16, tag="wfc")
    nc.sync.dma_start(wfc_sb, io["wfct"].rearrange("(cc p) m -> p cc m", p=128))
    suvu_sb = const.tile([128, 4], f32, tag="suvu")
    nc.sync.dma_start(suvu_sb, io["suvu"])
    suvv_sb = const.tile([128, 4], f32, tag="suvv")
    nc.sync.dma_start(suvv_sb, io["suvv"])
    wproj_sb = const.tile([128, 4, C], bf16, tag="wproj")
    nc.sync.dma_start(wproj_sb, io["wprojt"].rearrange("(fc p) m -> p fc m", p=128))
    lrs_sb = const.tile([128, 4 * C], bf16, tag="lrs")
    nc.sync.dma_start(lrs_sb, io["lrs"])
    ident_sb = const.tile([128, 128], bf16, tag="ident")
    make_identity(nc, ident_sb)
    ones164 = const.tile([1, D], f32, tag="ones164")
    nc.vector.memset(ones164, 1.0)

    qT_sb = const.tile([128, NT], bf16, tag="qT")
    kT_sb = const.tile([128, NT], bf16, tag="kT")
    v_sb = const.tile([128, 16, 2 * (D + 1)], bf16, tag="v")
    yT_sb = const.tile([128, NT], bf16, tag="yT")
    h1_sb = const.tile([128, 2, C], f32, tag="h1")

    # ---- phase 1a: q/k projections with fused rope + head-norm + sqk scale
    for w_sb, out_sb in ((wq_sb, qT_sb), (wk_sb, kT_sb)):
        for ntc in range(4):
            psq = ps.tile([128, 512], f32, tag="mm", bufs=3)
            for cc in range(8):
                nc.tensor.matmul(psq, lhsT=w_sb[:, cc, :],
                                 rhs=xt_sb[:, cc, ts(ntc, 512)],
                                 start=cc == 0, stop=cc == 7)
            qrot = tmp.tile([128, 512], f32, tag="qrot")
            nc.vector.tensor_mul(qrot, psq, rope_sb[:, ts(ntc, 512)])
            sq = tmp.tile([128, 512], bf16, tag="qsq")
            nc.vector.tensor_mul(sq, qrot, qrot)
            ssq = ps.tile([2, 512], f32, tag="aux")
            nc.tensor.matmul(ssq, lhsT=ones2_sb, rhs=sq, start=True, stop=True)
            srt = tmp.tile([2, 512], f32, tag="qsmall")
            nc.scalar.activation(srt, ssq, AF.Sqrt)
            rn = tmp.tile([2, 512], f32, tag="qsmall")
            nc.vector.reciprocal(rn, srt)
            bc = ps.tile([128, 512], f32, tag="aux")
            nc.tensor.matmul(bc, lhsT=normind_sb, rhs=rn, start=True, stop=True)
            nc.vector.tensor_mul(out_sb[:, ts(ntc, 512)], qrot, bc)

    # ---- phase 1b: v in [tok, head*(D+1)] layout with ones column
    nc.vector.memset(v_sb[:, :, D:D + 1], 1.0)
    nc.vector.memset(v_sb[:, :, 2 * D + 1:2 * D + 2], 1.0)
    for tci in range(16):
        psv = ps.tile([128, 128], f32, tag="mm", bufs=3)
        for cc in range(8):
            nc.tensor.matmul(psv, lhsT=xt_sb[:, cc, ts(tci, 128)],
                             rhs=wv_sb[:, cc, :], start=cc == 0, stop=cc == 7)
        for hi in range(HPC):
            nc.scalar.copy(v_sb[:, tci, hi * (D + 1):hi * (D + 1) + D],
                           psv[:, hi * D:(hi + 1) * D])

    _ACT_CHAIN = [None, None]
    cc1 = None

    def _wo_and_rs(b):
        """partial att_c_proj for batch b's token rows, then row-split RS."""
        nonlocal cc1
        for tci in range(b * 8, b * 8 + 8):
            for ncc in range(2):
                pso = ps.tile([128, 512], f32, tag="mm", bufs=3,
                              name=f"pso_{tci}_{ncc}")
                nc.tensor.matmul(pso, lhsT=yT_sb[:, ts(tci, 128)],
                                 rhs=wo_sb[:, ts(ncc, 512)], start=True, stop=True)
                ha = tmp.tile([128, 512], bf16, tag="ha", name=f"ha_{tci}_{ncc}")
                nc.vector.tensor_copy(ha, pso)
                nc.sync.dma_start(hatt_part[ts(tci, 128), ts(ncc, 512)], ha)
        if mock_cc:
            cc1 = nc.sync.dma_start(hatt_rs[b][:], hatt_part[b * T:b * T + 128, :])
        else:
            cc1 = nc.gpsimd.collective_compute(
                "ReduceScatter", mybir.AluOpType.add, replica_groups=RG,
                ins=[hatt_part[b * T:(b + 1) * T, :]], outs=[hatt_rs[b][:]])

    # ---- phase 1c: attention per (batch, head)
    # Two passes over the (qc, kc) chunks per (b, head): pass 0 computes all
    # exp() chunks (Exp LUT loaded once), pass 1 recomputes scores on PE and
    # does Sigmoid + gating + PV (Sigmoid LUT loaded once) — avoids per-chunk
    # ACT table thrash (~1.3us per switch). Chunks on/below the causal
    # diagonal are width-trimmed to their live columns.
    def _attn_chunks(qc):
        n_kc = min(8, 4 * qc + 4)
        out = []
        for kc in range(n_kc):
            m = kc - 4 * qc
            off = max(0, 128 * m)   # first live column within the qc chunk
            out.append((kc, m, off, 512 - off))
        return out

    for b in range(B):
        for hi in range(HPC):
            dsl = ds(hi * D, D)
            y_aug = [ps.tile([D + 1, 512], f32, tag="y", name=f"y_{b}_{hi}_{qc2}")
                     for qc2 in range(2)]
            e_tiles = {}
            exp_insts = []
            for qc in range(2):
                for kc, m, off, w in _attn_chunks(qc):
                    s_ps = ps.tile([128, 512], f32, tag="mm", bufs=3)
                    nc.tensor.matmul(
                        s_ps[:, :w],
                        lhsT=kT_sb[dsl, ds(b * T + kc * 128, 128)],
                        rhs=qT_sb[dsl, ds(b * T + qc * 512 + off, w)],
                        start=True, stop=True)
                    e_sb = tmp.tile([128, 512], bf16, tag="e", bufs=14,
                                    name=f"e_{b}_{hi}_{qc}_{kc}")
                    ei = nc.scalar.activation(e_sb[:, :w], s_ps[:, :w], AF.Exp,
                                              scale=SM_SCALE)
                    exp_insts.append(ei)
                    e_tiles[(qc, kc)] = e_sb
            # ACT LUT grouping: first exp of this group after last sigmoid of
            # the previous group; first sigmoid after last exp of this group.
            if _ACT_CHAIN[0] is not None:
                _add_dep(exp_insts[0].ins, _ACT_CHAIN[0].ins,
                         reason="ACT table grouping: exp group after sigmoids")
            first_sig = [None]
            for qc in range(2):
                first = True
                chunks = _attn_chunks(qc)
                for kc, m, off, w in chunks:
                    s_ps = ps.tile([128, 512], f32, tag="mm", bufs=3)
                    nc.tensor.matmul(
                        s_ps[:, :w],
                        lhsT=kT_sb[dsl, ds(b * T + kc * 128, 128)],
                        rhs=qT_sb[dsl, ds(b * T + qc * 512 + off, w)],
                        start=True, stop=True)
                    g_sb = tmp.tile([128, 512], bf16, tag="g")
                    gi = nc.scalar.activation(g_sb[:, :w], s_ps[:, :w], AF.Sigmoid,
                                              scale=gatesc_sb[:, hi:hi + 1],
                                              bias=gatebi_sb[:, hi:hi + 1])
                    if first_sig[0] is None:
                        first_sig[0] = gi
                        _add_dep(gi.ins, exp_insts[-1].ins,
                                 reason="ACT table grouping: sigmoids after exps")
                    _ACT_CHAIN[0] = gi
                    p_sb = tmp.tile([128, 512], bf16, tag="p")
                    nc.vector.tensor_mul(p_sb[:, :w], e_tiles[(qc, kc)][:, :w],
                                         g_sb[:, :w])
                    if m >= 0:
                        nc.vector.tensor_mul(p_sb[:, :w], p_sb[:, :w],
                                             masks_sb[:, m, off:512])
                    nc.tensor.matmul(
                        y_aug[qc][:, off:512],
                        lhsT=v_sb[:, b * 8 + kc, ds(hi * (D + 1), D + 1)],
                        rhs=p_sb[:, :w],
                        start=first, stop=kc == chunks[-1][0])
                    first = False
            # renormalize: yT = y[:D] / y[D]
            for qc in range(2):
                rcp = tmp.tile([1, 512], f32, tag="rpool")
                nc.vector.reciprocal(rcp, y_aug[qc][D:D + 1, :])
                rb = ps.tile([D, 512], f32, tag="aux")
                nc.tensor.matmul(rb, lhsT=ones164, rhs=rcp, start=True, stop=True)
                rb_sb = tmp.tile([D, 512], f32, tag="rpool")
                nc.vector.tensor_copy(rb_sb, rb)
                nc.vector.tensor_mul(
                    yT_sb[dsl, ds(b * T + qc * 512, 512)], y_aug[qc][:D, :], rb_sb)
        _wo_and_rs(b)

    # ---- residual update #1 (local 256 rows) + transpose to h1t
    h1t_tiles = []
    for cc in range(8):
        h1t_tiles.append(const.tile([128, ROWS], bf16, tag=f"h1t{cc}",
                                    name=f"h1t_sb{cc}"))
    for r in range(2):
        ha_bf = tmp.tile([128, C], bf16, tag="res_in")
        nc.sync.dma_start(ha_bf, hatt_rs[r][:])
        hloc_sb = tmp.tile([128, C], f32, tag="res_hloc")
        nc.sync.dma_start(hloc_sb, io["hloc"][ts(r, 128), :])
        ra = _rnorm_of(nc, tmp, hloc_sb)
        _residual_update(nc, tmp, h1_sb[:, r, :], hloc_sb, ra, ha_bf,
                         lrs_sb[:, 0:C], lrs_sb[:, C:2 * C])
        h1b = tmp.tile([128, C], bf16, tag="res_bf")
        nc.scalar.copy(h1b, h1_sb[:, r, :])
        for cc in range(8):
            tps = ps.tile([128, 128], bf16, tag="aux")
            nc.tensor.transpose(tps, h1b[:, ts(cc, 128)], ident_sb)
            nc.scalar.copy(h1t_tiles[cc][:, ts(r, 128)], tps)
    for cc in range(8):
        nc.sync.dma_start(h1t_loc[ts(cc, 128), :], h1t_tiles[cc])

    # ---- AllGather of h1t (partition axis) -> (8*C, ROWS) = [g, c, tok]
    if mock_cc:
        for g_ in range(NC_):
            cc2 = nc.sync.dma_start(h1t_all[g_ * C:(g_ + 1) * C, :], h1t_loc[:])
    else:
        cc2 = nc.gpsimd.collective_compute(
            "AllGather", mybir.AluOpType.bypass, replica_groups=RG,
            ins=[h1t_loc[:]], outs=[h1t_all[:]])

    # ---- load x1 = h1^T full (C on partitions, tokens free; token = g*ROWS+t)
    x1_sb = const.tile([128, 8, NT], bf16, tag="xmat")  # reuses xt slot
    h1t_view = h1t_all.rearrange("(g cc p) t -> p cc g t", g=NC_, p=128)
    x1_view = x1_sb.rearrange("p cc (g t) -> p cc g t", g=NC_)
    for cc in range(8):
        nc.sync.dma_start(x1_view[:, cc], h1t_view[:, cc])

    # ---- MLP: u/v matmuls + silu + partial c_proj
    xm_tiles = []
    for ntc in range(4):
        xm4 = tmp.tile([128, 4, 512], bf16, tag="xm4", bufs=4,
                       name=f"xm4_{ntc}")
        for fc in range(4):
            psu = ps.tile([128, 512], f32, tag="mm", bufs=3)
            psv2 = ps.tile([128, 512], f32, tag="mm2", bufs=1)
            for cc in range(8):
                nc.tensor.matmul(psu, lhsT=wfc_sb[:, cc, ds(fc * 128, 128)],
                                 rhs=x1_sb[:, cc, ts(ntc, 512)],
                                 start=cc == 0, stop=cc == 7)
            for cc in range(8):
                nc.tensor.matmul(psv2, lhsT=wfc_sb[:, cc, ds(FH + fc * 128, 128)],
                                 rhs=x1_sb[:, cc, ts(ntc, 512)],
                                 start=cc == 0, stop=cc == 7)
            sv = tmp.tile([128, 512], bf16, tag="silu")
            if sim_safe:
                sg = tmp.tile([128, 512], bf16, tag="sg")
                nc.scalar.activation(sg, psv2, AF.Sigmoid,
                                     scale=suvv_sb[:, fc:fc + 1])
                nc.vector.scalar_tensor_tensor(
                    sv, in0=psv2, scalar=suvv_sb[:, fc:fc + 1],
                    in1=sg, op0=mybir.AluOpType.mult, op1=mybir.AluOpType.mult)
            else:
                nc.scalar.activation(sv, psv2, AF.Silu,
                                     scale=suvv_sb[:, fc:fc + 1])
            nc.vector.scalar_tensor_tensor(
                xm4[:, fc, :], in0=psu, scalar=suvu_sb[:, fc:fc + 1],
                in1=sv, op0=mybir.AluOpType.mult, op1=mybir.AluOpType.mult)
        xm_tiles.append(xm4)
    for ncc in range(2):
        for tci in range(16):
            psp = ps.tile([128, 512], f32, tag="mm", bufs=3,
                          name=f"psp_{ncc}_{tci}")
            xm4 = xm_tiles[tci // 4]
            tsub = tci % 4
            for fc in range(4):
                nc.tensor.matmul(psp, lhsT=xm4[:, fc, ts(tsub, 128)],
                                 rhs=wproj_sb[:, fc, ts(ncc, 512)],
                                 start=fc == 0, stop=fc == 3)
            hm = tmp.tile([128, 512], bf16, tag="hm", name=f"hm_{ncc}_{tci}")
            nc.vector.tensor_copy(hm, psp)
            nc.sync.dma_start(hmlp_part[ncc][ts(tci, 128), :], hm)
        # ---- ReduceScatter #2 (column half ncc)
        if mock_cc:
            nc.sync.dma_start(hmlp_rs[ncc][:], hmlp_part[ncc][0:ROWS, :])
        else:
            nc.gpsimd.collective_compute(
                "ReduceScatter", mybir.AluOpType.add, replica_groups=RG,
                ins=[hmlp_part[ncc][:]], outs=[hmlp_rs[ncc][:]])

    # ---- residual update #2 -> output (h1 already unit-norm: ra=None)
    for r in range(2):
        hm_bf = tmp.tile([128, C], bf16, tag="res_in")
        nc.sync.dma_start(hm_bf[:, 0:512], hmlp_rs[0][ts(r, 128), :])
        nc.sync.dma_start(hm_bf[:, 512:1024], hmlp_rs[1][ts(r, 128), :])
        out_f = tmp.tile([128, C], f32, tag="res_out", bufs=1)
        _residual_update(nc, tmp, out_f, h1_sb[:, r, :], None, hm_bf,
                         lrs_sb[:, 2 * C:3 * C], lrs_sb[:, 3 * C:4 * C])
        nc.sync.dma_start(io["out"][ts(r, 128), :], out_f)



_CACHE = {}


def _get_built(mock_cc=False, sim_safe=False):
    key = ("nc", mock_cc, sim_safe)
    if key in _CACHE:
        return _CACHE[key]
    nc = bacc.Bacc(get_trn_type() or "TRN2", target_bir_lowering=False,
                   debug=False, num_devices=NC_)
    io = {}
    for name, shape, dt in _INPUT_SPECS:
        io[name] = nc.dram_tensor(name, list(shape), dt, kind="ExternalInput").ap()
    io["out"] = nc.dram_tensor("out", [ROWS, C], f32, kind="ExternalOutput").ap()
    with tile.TileContext(nc) as tc:
        _build_kernel(tc, io, mock_cc=mock_cc, sim_safe=sim_safe)
    nc.compile()
    _CACHE[key] = nc
    return nc


def kernel(**inputs) -> np.ndarray:
    rope, perm128, masks = _host_tables()
    in_maps = [_core_inputs(g, inputs, rope, perm128, masks) for g in range(NC_)]
    nc = _get_built(
        sim_safe=bool(int(os.environ.get("KERNEL_SIM_SAFE", "0"))))
    trace = bool(int(os.environ.get("KERNEL_TRACE", "0")))
    res = run_bass_kernel_spmd(nc, in_maps, core_ids=list(range(NC_)), trace=trace)
    if trace and res.exec_time_ns is not None:
        print(f"HW exec time: {res.exec_time_ns} ns")
        _CACHE["exec_time_ns"] = res.exec_time_ns
        _CACHE["trace"] = res.instructions_and_trace
    out = np.zeros((NT, C), np.float32)
    for g in range(NC_):
        og = res.results[g]["out"]
        out[g * 128:(g + 1) * 128] = og[0:128]
        out[T + g * 128:T + (g + 1) * 128] = og[128:256]
    return out.reshape(B, T, C).astype(np.float32)


if __name__ == "__main__":
    rng = np.random.default_rng(0)
    fake = {
        "h": rng.standard_normal((B, T, C), dtype=np.float32),
        "Wq": rng.standard_normal((C, C), dtype=np.float32) * 0.02,
        "Wk": rng.standard_normal((C, C), dtype=np.float32) * 0.02,
        "Wv": rng.standard_normal((C, C), dtype=np.float32) * 0.02,
        "Wo": rng.standard_normal((C, C), dtype=np.float32) * 0.02,
        "Wfc": rng.standard_normal((8 * C, C), dtype=np.float32) * 0.02,
        "Wproj": rng.standard_normal((C, 4 * C), dtype=np.float32) * 0.02,
        "sqk": BASE_SCALE * np.ones(C, np.float32),
        "suv": np.ones(8 * C, np.float32),
        "attn_alpha": BASE_SCALE * np.ones(C, np.float32),
        "mlp_alpha": BASE_SCALE * np.ones(C, np.float32),
        "thr_c": 1.6 * np.ones(H, np.float32),
        "stp": 10.0 * np.ones(H, np.float32),
    }
    out = kernel(**fake)
    print("out", out.shape, out.dtype, np.abs(out).mean())



# revision 14
# speedup vs baseline: 1.4487x; 1.4487x over previous
"""Self-contained Trainium2 Bass kernel for nn_Block_86028194939235 (sparse_attention).

nGPT-style block: gated-softmax causal attention + 8C MLP, B=2 T=1024 C=1024 H=16.

Sharding (8 cores, hardcoded):
  - attention: heads sharded (2 heads/core); partial att_c_proj output
    ReduceScatter(add) over token rows -> each core owns 256 rows.
  - residual/norm work: sharded over the 256 local rows.
  - MLP: token-sharded — every core runs the FULL 8C MLP for its own 256
    rows (fp8 Wfc/Wproj resident in SBUF, streamed in during attention).
    No AllGather, no second ReduceScatter.
  - final output: each core returns its 256 rows; host concatenates.

fp8(e4m3) for QKV projections and both MLP GEMMs (justnorm makes global
scales vanish); bf16 for attention core; f32 residual/norm math.
"""
import math
import os

import numpy as np
import ml_dtypes

import concourse.bass as bass
import concourse.bacc as bacc
import concourse.mybir as mybir
import concourse.tile as tile
from concourse.bass import ts, ds
from concourse.bass_utils import run_bass_kernel_spmd
from concourse.masks import make_identity
from concourse.tile import add_dep_helper as _add_dep
from concourse._compat import with_exitstack, get_trn_type
from contextlib import ExitStack

NC_ = 8
B, T, C, H, D = 2, 1024, 1024, 16, 64
NT = B * T                 # 2048 tokens
HPC = H // NC_             # 2 heads per core
GD = HPC * D               # 128
ROWS = NT // NC_           # 256 rows per core
FHID = 8 * C               # 8192 full mlp hidden (u+v)
NPAIR = 32                 # 32 (u,v) 128-row pairs
BASE_SCALE = 0.03125
SM_SCALE = math.sqrt(D)    # 8.0

# fp8 quantization scales (powers of two; all wash out through justnorm
# or are folded into suvu/suvv)
XS = 16.0      # h (attention input)
WS = 512.0     # Wq/Wk/Wv
FS = 1024.0    # Wfc
X1S = 128.0    # x1 (mlp input, unit-norm rows)
XMS = 8.0      # x_mlp (u*silu(v))
PS_ = 1024.0   # Wproj

bf16 = mybir.dt.bfloat16
f32 = mybir.dt.float32
fp8 = mybir.dt.float8e4
nbf = ml_dtypes.bfloat16
nf8 = ml_dtypes.float8_e4m3fn
AF = mybir.ActivationFunctionType
MUL = mybir.AluOpType.mult


def _to_bf(x):
    return np.ascontiguousarray(np.asarray(x, np.float32).astype(nbf))


def _to_f8(x, scale):
    a = np.asarray(x, np.float32) * scale
    return np.ascontiguousarray(np.clip(a, -240.0, 240.0).astype(nf8))


def _f32(x):
    return np.ascontiguousarray(np.asarray(x, np.float32))


# ---------------------------------------------------------------- host tables
def _sinusoidal_embeddings(n, d):
    pos = np.arange(n, dtype=np.float32)[:, None]
    div = np.exp(np.arange(0, d, 2, dtype=np.float32) * (-math.log(10000.0) / d))
    emb = np.zeros((n, d), dtype=np.float32)
    emb[:, 0::2] = np.sin(pos * div)
    emb[:, 1::2] = np.cos(pos * div)
    return emb


def _host_tables():
    emb = _sinusoidal_embeddings(T, D)
    R = np.zeros((D, T), dtype=np.float32)
    for i in range(D // 2):
        R[2 * i, :] = -emb[:, 32 + i]
        R[2 * i + 1, :] = emb[:, i]
    rope = np.tile(np.tile(R, (HPC, 1)), (1, B))          # (128, 2048)
    perm64 = np.arange(D).reshape(-1, 2)[:, ::-1].reshape(-1)
    perm128 = np.concatenate([perm64, perm64 + D])
    masks = np.zeros((4, 128, 512), dtype=np.float32)
    for m in range(4):
        masks[m] = (np.arange(512)[None, :] - 128 * m - np.arange(128)[:, None]) >= 0
    return rope, perm128, masks


def _mlp_tables(inp):
    """Full (unsharded) MLP weights, fp8, identical on every core."""
    Wfc = _f32(inp["Wfc"])                      # (8C, C)
    wfct = Wfc.T                                # (C, 8C): cols = hidden rows
    paired = np.empty((C, FHID), np.float32)
    for p in range(NPAIR):
        paired[:, p * 256:p * 256 + 128] = wfct[:, p * 128:(p + 1) * 128]
        paired[:, p * 256 + 128:p * 256 + 256] = \
            wfct[:, 4 * C + p * 128:4 * C + (p + 1) * 128]
    suv = _f32(inp["suv"]) * math.sqrt(C)
    suvu = suv[:4 * C].reshape(NPAIR, 128).T * (XMS / (X1S * FS))   # (128, 32)
    suvv = suv[4 * C:].reshape(NPAIR, 128).T * (1.0 / (X1S * FS))
    wprojt = _f32(inp["Wproj"]).T               # (4C, C): rows = hidden
    return (_to_f8(paired, FS), _f32(suvu), _f32(suvv), _to_f8(wprojt, PS_))


def _core_inputs(g, inp, rope, perm128, masks, mlp_tabs):
    h = _f32(inp["h"]).reshape(NT, C)
    gd = slice(g * GD, (g + 1) * GD)
    sqk_s = _f32(inp["sqk"])[gd] * (1.0 / BASE_SCALE)
    normind = np.zeros((2, GD), np.float32)
    normind[np.arange(GD) // D, np.arange(GD)] = sqk_s
    ones2 = np.zeros((GD, 2), np.float32)
    ones2[np.arange(GD), np.arange(GD) // D] = 1.0
    stp = _f32(inp["stp"])
    thr = _f32(inp["thr_c"])
    gatesc = np.zeros((128, 2), np.float32)
    gatebi = np.zeros((128, 2), np.float32)
    for hi in range(HPC):
        hh = HPC * g + hi
        gatesc[:, hi] = SM_SCALE * stp[hh]
        gatebi[:, hi] = -stp[hh] * thr[hh]
    lr1 = np.abs(_f32(inp["attn_alpha"]) * (0.05 / BASE_SCALE))
    lr2 = np.abs(_f32(inp["mlp_alpha"]) * (0.05 / BASE_SCALE))
    lrs = np.concatenate([1.0 - lr1, lr1, 1.0 - lr2, lr2])
    wfcq, suvu, suvv, wprojq = mlp_tabs
    return {
        "xtq": _to_f8(h.T, XS),
        "hloc": _f32(np.concatenate(
            [h[g * 128:(g + 1) * 128], h[T + g * 128:T + (g + 1) * 128]], 0)),
        "wqt": _to_f8(_f32(inp["Wq"])[gd][perm128].T, WS),
        "wkt": _to_f8(_f32(inp["Wk"])[gd][perm128].T, WS),
        "wvt": _to_f8(_f32(inp["Wv"])[gd].T, WS),
        "wot": _to_bf(_f32(inp["Wo"])[:, gd].T),
        "rope": _to_bf(rope),
        "normind": _f32(normind),
        "ones2": _to_bf(ones2),
        "masks": _to_f8(masks, 1.0),
        "gatesc": _f32(gatesc),
        "gatebi": _f32(gatebi),
        "wfcq": wfcq,
        "suvu": suvu,
        "suvv": suvv,
        "wprojq": wprojq,
        "lrs": np.ascontiguousarray(
            np.broadcast_to(lrs.reshape(1, 4 * C), (128, 4 * C)).astype(nbf)),
    }


_INPUT_SPECS = [
    ("xtq", (C, NT), fp8),
    ("hloc", (ROWS, C), f32),
    ("wqt", (C, GD), fp8),
    ("wkt", (C, GD), fp8),
    ("wvt", (C, GD), fp8),
    ("wot", (GD, C), bf16),
    ("rope", (GD, NT), bf16),
    ("normind", (2, GD), f32),
    ("ones2", (GD, 2), bf16),
    ("masks", (4, 128, 512), fp8),
    ("gatesc", (128, 2), f32),
    ("gatebi", (128, 2), f32),
    ("wfcq", (C, FHID), fp8),
    ("suvu", (128, NPAIR), f32),
    ("suvv", (128, NPAIR), f32),
    ("wprojq", (4 * C, C), fp8),
    ("lrs", (128, 4 * C), bf16),
]


# ---------------------------------------------------------------- device code
def _residual_update(nc, tmp, out_f32, a_sb, ra, upd_bf, lrm_ap, lrv_ap):
    """out = justnorm(lrm*(ra*a) + lrv*justnorm(upd)).

    a_sb: [128, C] f32 (A, pre-norm); ra: [128,1] f32 reciprocal-norm of a
          (pass None to treat a as already unit-norm).
    upd_bf: [128, C] bf16 (update branch, pre-norm; any global scale).
    """
    sq = tmp.tile([128, C], bf16, tag="res_sq")
    ssb = tmp.tile([128, 1], f32, tag="res_ss")
    nc.scalar.activation(sq, upd_bf, AF.Square, accum_out=ssb)
    srt = tmp.tile([128, 1], f32, tag="res_srt")
    nc.scalar.activation(srt, ssb, AF.Sqrt)
    rb = tmp.tile([128, 1], f32, tag="res_rb")
    nc.vector.reciprocal_approx_fast(rb, srt)

    t1 = tmp.tile([128, C], f32, tag="res_t1", bufs=1)
    if ra is None:
        nc.vector.tensor_mul(t1, a_sb, lrm_ap)
    else:
        nc.vector.scalar_tensor_tensor(
            t1, in0=a_sb, scalar=ra, in1=lrm_ap, op0=MUL, op1=MUL)
    t2 = tmp.tile([128, C], f32, tag="res_t2", bufs=1)
    nc.vector.scalar_tensor_tensor(
        t2, in0=upd_bf, scalar=rb, in1=lrv_ap, op0=MUL, op1=MUL)
    nc.vector.tensor_add(t1, t1, t2)
    sq2 = tmp.tile([128, C], bf16, tag="res_sq")
    ss2 = tmp.tile([128, 1], f32, tag="res_ss")
    nc.scalar.activation(sq2, t1, AF.Square, accum_out=ss2)
    srt2 = tmp.tile([128, 1], f32, tag="res_srt")
    nc.scalar.activation(srt2, ss2, AF.Sqrt)
    rs = tmp.tile([128, 1], f32, tag="res_rb")
    nc.vector.reciprocal_approx_fast(rs, srt2)
    nc.vector.tensor_scalar_mul(out_f32, t1, rs)


def _rnorm_of(nc, tmp, x_sb):
    """reciprocal L2 norm over free dim: [128, C] f32 -> [128,1] f32."""
    sq = tmp.tile([128, C], bf16, tag="res_sq")
    ssb = tmp.tile([128, 1], f32, tag="res_ss")
    nc.scalar.activation(sq, x_sb, AF.Square, accum_out=ssb)
    srt = tmp.tile([128, 1], f32, tag="res_srt")
    nc.scalar.activation(srt, ssb, AF.Sqrt)
    r = tmp.tile([128, 1], f32, tag="res_ra")
    nc.vector.reciprocal_approx_fast(r, srt)
    return r


@with_exitstack
def _build_kernel(ctx: ExitStack, tc: tile.TileContext, io: dict, mock_cc=False,
                  sim_safe=False, debug_dump=False):
    nc = tc.nc
    RG = [[i for i in range(NC_)]]

    # internal DRAM for the attention ReduceScatter
    hatt_part = nc.dram_tensor("hatt_part", [NT, C], bf16, kind="Internal").ap()
    hatt_rs = [nc.dram_tensor(f"hatt_rs{i}", [128, C], bf16, kind="Internal").ap()
               for i in range(2)]

    const = ctx.enter_context(tc.tile_pool(name="const", bufs=1))
    tmp = ctx.enter_context(tc.tile_pool(name="tmp", bufs=2))
    ps = ctx.enter_context(tc.tile_pool(name="ps", bufs=2, space="PSUM"))
    wstream = ctx.enter_context(tc.tile_pool(name="wstream", bufs=4))

    # ---- load constants / weights to SBUF (sync queue)
    wq_sb = const.tile([128, 8, GD], fp8, tag="wq")
    nc.sync.dma_start(wq_sb, io["wqt"].rearrange("(cc p) m -> p cc m", p=128))
    xt_sb = const.tile([128, 8, NT], fp8, tag="xmat")
    xt_view = io["xtq"].rearrange("(cc p) t -> p cc t", p=128)
    for cc in range(8):
        nc.sync.dma_start(xt_sb[:, cc], xt_view[:, cc])
    wk_sb = const.tile([128, 8, GD], fp8, tag="wk")
    nc.sync.dma_start(wk_sb, io["wkt"].rearrange("(cc p) m -> p cc m", p=128))
    wv_sb = const.tile([128, 8, GD], fp8, tag="wv")
    nc.sync.dma_start(wv_sb, io["wvt"].rearrange("(cc p) m -> p cc m", p=128))
    wo_sb = const.tile([128, C], bf16, tag="wo")
    nc.sync.dma_start(wo_sb, io["wot"])
    rope_sb = const.tile([128, NT], bf16, tag="rope")
    nc.sync.dma_start(rope_sb, io["rope"])
    normind_sb = const.tile([2, GD], f32, tag="normind")
    nc.sync.dma_start(normind_sb, io["normind"])
    ones2_sb = const.tile([GD, 2], bf16, tag="ones2")
    nc.sync.dma_start(ones2_sb, io["ones2"])
    masks_sb = const.tile([128, 4, 512], fp8, tag="masks")
    nc.sync.dma_start(masks_sb, io["masks"].rearrange("m p q -> p m q"))
    gatesc_sb = const.tile([128, 2], f32, tag="gatesc")
    nc.sync.dma_start(gatesc_sb, io["gatesc"])
    gatebi_sb = const.tile([128, 2], f32, tag="gatebi")
    nc.sync.dma_start(gatebi_sb, io["gatebi"])
    suvu_sb = const.tile([128, NPAIR], f32, tag="suvu")
    nc.sync.dma_start(suvu_sb, io["suvu"])
    suvv_sb = const.tile([128, NPAIR], f32, tag="suvv")
    nc.sync.dma_start(suvv_sb, io["suvv"])
    lrs_sb = const.tile([128, 4 * C], bf16, tag="lrs")
    nc.sync.dma_start(lrs_sb, io["lrs"])
    ident_sb = const.tile([128, 128], bf16, tag="ident")
    make_identity(nc, ident_sb)
    ones164 = const.tile([34, D], f32, tag="ones164")
    nc.vector.memset(ones164, 1.0)

    # ---- full MLP weights (fp8, resident): stream on the ACT hwdge queue so
    # the sync queue keeps feeding attention. Issued first on that engine.
    wfcq_sb = const.tile([128, 8, FHID], fp8, tag="wfcq")
    wfcq_view = io["wfcq"].rearrange("(cc p) m -> p cc m", p=128)
    for cc in range(8):
        nc.scalar.dma_start(wfcq_sb[:, cc], wfcq_view[:, cc])

    qT_sb = const.tile([128, NT], bf16, tag="qT")
    kT_sb = const.tile([128, NT], bf16, tag="kT")
    v_sb = const.tile([128, 16, 2 * (D + 1)], bf16, tag="v")
    yT_sb = const.tile([128, NT], bf16, tag="yT")
    h1_sb = const.tile([128, 2, C], f32, tag="h1")
    x1q_sb = const.tile([128, 8, ROWS], fp8, tag="xmat")
    xmq_sb = const.tile([128, NPAIR, ROWS], fp8, tag="xmq")

    # ---- phase 1a: q/k projections with fused rope + head-norm + sqk scale
    for w_sb, out_sb in ((wq_sb, qT_sb), (wk_sb, kT_sb)):
        for ntc in range(4):
            psq = ps.tile([128, 512], f32, tag="mm", bufs=3)
            for cc in range(8):
                nc.tensor.matmul(psq, lhsT=w_sb[:, cc, :],
                                 rhs=xt_sb[:, cc, ts(ntc, 512)],
                                 start=cc == 0, stop=cc == 7)
            qrot = tmp.tile([128, 512], f32, tag="qrot")
            nc.vector.tensor_mul(qrot, psq, rope_sb[:, ts(ntc, 512)])
            sq = tmp.tile([128, 512], bf16, tag="qsq")
            nc.vector.tensor_mul(sq, qrot, qrot)
            ssq = ps.tile([2, 512], f32, tag="aux")
            nc.tensor.matmul(ssq, lhsT=ones2_sb, rhs=sq, start=True, stop=True)
            srtq = tmp.tile([2, 512], f32, tag="rpool")
            nc.scalar.activation(srtq, ssq, AF.Sqrt)
            rn = tmp.tile([2, 512], f32, tag="rpool")
            nc.vector.reciprocal_approx_fast(rn, srtq)
            bc = ps.tile([128, 512], f32, tag="aux")
            nc.tensor.matmul(bc, lhsT=normind_sb, rhs=rn, start=True, stop=True)
            nc.vector.tensor_mul(out_sb[:, ts(ntc, 512)], qrot, bc)

    # ---- phase 1b: v in [tok, head*(D+1)] layout with ones column
    nc.vector.memset(v_sb[:, :, D:D + 1], 1.0)
    nc.vector.memset(v_sb[:, :, 2 * D + 1:2 * D + 2], 1.0)
    for tci in range(16):
        psv = ps.tile([128, 128], f32, tag="mm", bufs=3)
        for cc in range(8):
            nc.tensor.matmul(psv, lhsT=xt_sb[:, cc, ts(tci, 128)],
                             rhs=wv_sb[:, cc, :], start=cc == 0, stop=cc == 7)
        # strided copy: both heads at once into the (D+1)-strided layout
        vout = v_sb[:, tci].rearrange("p (h e) -> p h e", h=2)[:, :, 0:D]
        vin = psv.rearrange("p (h d) -> p h d", h=2)
        nc.vector.tensor_copy(vout, vin)

    _ACT_CHAIN = [None, None]
    cc1 = None

    def _wo_and_rs(b):
        """partial att_c_proj for batch b's token rows, then row-split RS."""
        nonlocal cc1
        for tci in range(b * 8, b * 8 + 8):
            for ncc in range(2):
                pso = ps.tile([128, 512], f32, tag="mm", bufs=3,
                              name=f"pso_{tci}_{ncc}")
                nc.tensor.matmul(pso, lhsT=yT_sb[:, ts(tci, 128)],
                                 rhs=wo_sb[:, ts(ncc, 512)], start=True, stop=True)
                ha = tmp.tile([128, 512], bf16, tag="ha", name=f"ha_{tci}_{ncc}")
                nc.vector.tensor_copy(ha, pso)
                nc.sync.dma_start(hatt_part[ts(tci, 128), ts(ncc, 512)], ha)
        if mock_cc:
            cc1 = nc.sync.dma_start(hatt_rs[b][:], hatt_part[b * T:b * T + 128, :])
        else:
            cc1 = nc.gpsimd.collective_compute(
                "ReduceScatter", mybir.AluOpType.add, replica_groups=RG,
                ins=[hatt_part[b * T:(b + 1) * T, :]], outs=[hatt_rs[b][:]])

    # ---- phase 1c: attention per (batch, head)
    # Two passes over the (qc, kc) chunks per (b, head): pass 0 computes all
    # exp() chunks (Exp LUT loaded once), pass 1 recomputes scores on PE and
    # does Sigmoid + gating + PV (Sigmoid LUT loaded once) — avoids per-chunk
    # ACT table thrash (~1.3us per switch). Chunks on/below the causal
    # diagonal are width-trimmed to their live columns.
    def _attn_chunks(qc):
        n_kc = min(8, 4 * qc + 4)
        out = []
        for kc in range(n_kc):
            m = kc - 4 * qc
            off = max(0, 128 * m)   # first live column within the qc chunk
            out.append((kc, m, off, 512 - off))
        return out

    for b in range(B):
        for hi in range(HPC):
            dsl = ds(hi * D, D)
            y_aug = [ps.tile([D + 1, 512], f32, tag="y", name=f"y_{b}_{hi}_{qc2}")
                     for qc2 in range(2)]
            e_tiles = {}
            exp_insts = []
            for qc in range(2):
                for kc, m, off, w in _attn_chunks(qc):
                    s_ps = ps.tile([128, 512], f32, tag="mm", bufs=3)
                    nc.tensor.matmul(
                        s_ps[:, :w],
                        lhsT=kT_sb[dsl, ds(b * T + kc * 128, 128)],
                        rhs=qT_sb[dsl, ds(b * T + qc * 512 + off, w)],
                        start=True, stop=True)
                    e_sb = tmp.tile([128, 512], bf16, tag="e", bufs=13,
                                    name=f"e_{b}_{hi}_{qc}_{kc}")
                    ei = nc.scalar.activation(e_sb[:, :w], s_ps[:, :w], AF.Exp,
                                              scale=SM_SCALE)
                    exp_insts.append(ei)
                    e_tiles[(qc, kc)] = e_sb
            # ACT LUT grouping: first exp of this group after last sigmoid of
            # the previous group; first sigmoid after last exp of this group.
            if _ACT_CHAIN[0] is not None:
                _add_dep(exp_insts[0].ins, _ACT_CHAIN[0].ins,
                         reason="ACT table grouping: exp group after sigmoids")
            first_sig = [None]
            for qc in range(2):
                first = True
                chunks = _attn_chunks(qc)
                for kc, m, off, w in chunks:
                    s_ps = ps.tile([128, 512], f32, tag="mm", bufs=3)
                    nc.tensor.matmul(
                        s_ps[:, :w],
                        lhsT=kT_sb[dsl, ds(b * T + kc * 128, 128)],
                        rhs=qT_sb[dsl, ds(b * T + qc * 512 + off, w)],
                        start=True, stop=True)
                    g_sb = tmp.tile([128, 512], bf16, tag="g")
                    gi = nc.scalar.activation(g_sb[:, :w], s_ps[:, :w], AF.Sigmoid,
                                              scale=gatesc_sb[:, hi:hi + 1],
                                              bias=gatebi_sb[:, hi:hi + 1])
                    if first_sig[0] is None:
                        first_sig[0] = gi
                        _add_dep(gi.ins, exp_insts[-1].ins,
                                 reason="ACT table grouping: sigmoids after exps")
                    _ACT_CHAIN[0] = gi
                    p_sb = tmp.tile([128, 512], bf16, tag="p")
                    nc.vector.tensor_mul(p_sb[:, :w], e_tiles[(qc, kc)][:, :w],
                                         g_sb[:, :w])
                    if m >= 0:
                        nc.vector.tensor_mul(p_sb[:, :w], p_sb[:, :w],
                                             masks_sb[:, m, off:512])
                    nc.tensor.matmul(
                        y_aug[qc][:, off:512],
                        lhsT=v_sb[:, b * 8 + kc, ds(hi * (D + 1), D + 1)],
                        rhs=p_sb[:, :w],
                        start=first, stop=kc == chunks[-1][0])
                    first = False
            # renormalize: yT = y[:D] / y[D]. Both qc denominators go into
            # one [2,512] tile so the custom-DVE reciprocal runs on a
            # multi-partition shape (single-partition [1,512] miscomputes on
            # HW).
            den2 = tmp.tile([34, 512], f32, tag="rpool")
            nc.scalar.copy(den2[0:1, :], y_aug[0][D:D + 1, :])
            nc.scalar.copy(den2[32:33, :], y_aug[1][D:D + 1, :])
            rcp2 = tmp.tile([34, 512], f32, tag="rpool")
            nc.vector.reciprocal_approx_fast(rcp2, den2)
            for qc in range(2):
                rb = ps.tile([D, 512], f32, tag="aux")
                nc.tensor.matmul(rb, lhsT=ones164[ds(32 * qc, 1), :],
                                 rhs=rcp2[ds(32 * qc, 1), :],
                                 start=True, stop=True)
                rb_sb = tmp.tile([D, 512], f32, tag="rpool")
                nc.vector.tensor_copy(rb_sb, rb)
                nc.vector.tensor_mul(
                    yT_sb[dsl, ds(b * T + qc * 512, 512)], y_aug[qc][:D, :], rb_sb)
        _wo_and_rs(b)

    # ---- residual update #1 (local 256 rows) + transpose to fp8 x1^T
    for r in range(2):
        ha_bf = tmp.tile([128, C], bf16, tag="res_in")
        nc.sync.dma_start(ha_bf, hatt_rs[r][:])
        hloc_sb = tmp.tile([128, C], f32, tag="res_hloc", bufs=1)
        nc.sync.dma_start(hloc_sb, io["hloc"][ts(r, 128), :])
        ra = _rnorm_of(nc, tmp, hloc_sb)
        _residual_update(nc, tmp, h1_sb[:, r, :], hloc_sb, ra, ha_bf,
                         lrs_sb[:, 0:C], lrs_sb[:, C:2 * C])
        # h1 scaled by X1S into bf16, then PE-transpose into fp8 x1^T tiles
        h1b = tmp.tile([128, C], bf16, tag="res_bf")
        nc.scalar.activation(h1b, h1_sb[:, r, :], AF.Copy, scale=X1S)
        for cc in range(8):
            tps = ps.tile([128, 128], bf16, tag="aux")
            nc.tensor.transpose(tps, h1b[:, ts(cc, 128)], ident_sb)
            nc.vector.tensor_copy(x1q_sb[:, cc, ts(r, 128)], tps)

    # ---- stream Wproj (fp8) on the sync queue in 8 half-MB chunks
    # (4 hc-slices each). Wave 0 of c_proj reads chunks 0..7 ascending; wave 1
    # reads 7..0 descending, so chunks 4..7 are still resident and only 0..3
    # are re-fetched (issued between the waves).
    wprojq_view = io["wprojq"].rearrange("(hc p) m -> p hc m", p=128)
    wp_cur = {}

    def _load_wp(wc, gen):
        wp = wstream.tile([128, 4, C], fp8, tag="wproj", name=f"wp_{wc}_{gen}")
        nc.sync.dma_start(wp, wprojq_view[:, ts(wc, 4)])
        wp_cur[wc] = wp

    for wc in range(8):
        _load_wp(wc, 0)

    # ---- MLP (token-sharded, full hidden on this core): fc + silu
    for pr in range(NPAIR):
        psu = ps.tile([128, ROWS], f32, tag="mm", bufs=3)
        psv2 = ps.tile([128, ROWS], f32, tag="mm", bufs=3)
        for cc in range(8):
            nc.tensor.matmul(psu, lhsT=wfcq_sb[:, cc, ds(pr * 256, 128)],
                             rhs=x1q_sb[:, cc, :], start=cc == 0, stop=cc == 7)
        for cc in range(8):
            nc.tensor.matmul(psv2, lhsT=wfcq_sb[:, cc, ds(pr * 256 + 128, 128)],
                             rhs=x1q_sb[:, cc, :], start=cc == 0, stop=cc == 7)
        sv = tmp.tile([128, ROWS], bf16, tag="silu")
        if sim_safe:
            sg = tmp.tile([128, ROWS], bf16, tag="sg")
            nc.scalar.activation(sg, psv2, AF.Sigmoid,
                                 scale=suvv_sb[:, pr:pr + 1])
            nc.vector.scalar_tensor_tensor(
                sv, in0=psv2, scalar=suvv_sb[:, pr:pr + 1],
                in1=sg, op0=MUL, op1=MUL)
        else:
            nc.scalar.activation(sv, psv2, AF.Silu,
                                 scale=suvv_sb[:, pr:pr + 1])
        nc.vector.scalar_tensor_tensor(
            xmq_sb[:, pr, :], in0=psu, scalar=suvu_sb[:, pr:pr + 1],
            in1=sv, op0=MUL, op1=MUL)

    # ---- c_proj: h_mlp[tok, C] = x_mlp^T @ Wproj^T, contraction over hidden.
    # Two sequential token waves (2 PSUM banks each, reusing the "y" slots);
    # wave r's residual update overlaps wave r+1's matmuls.
    for r in range(2):
        if r == 1:
            for wc in (3, 2, 1, 0):
                _load_wp(wc, 1)
        psp0 = ps.tile([128, 512], f32, tag="y", name=f"psp0_{r}")
        psp1 = ps.tile([128, 512], f32, tag="y", name=f"psp1_{r}")
        hcs = range(NPAIR) if r == 0 else range(NPAIR - 1, -1, -1)
        for i, hc in enumerate(hcs):
            wp = wp_cur[hc // 4]
            nc.tensor.matmul(psp0, lhsT=xmq_sb[:, hc, ts(r, 128)],
                             rhs=wp[:, hc % 4, 0:512],
                             start=i == 0, stop=i == NPAIR - 1)
            nc.tensor.matmul(psp1, lhsT=xmq_sb[:, hc, ts(r, 128)],
                             rhs=wp[:, hc % 4, 512:1024],
                             start=i == 0, stop=i == NPAIR - 1)
        # ---- residual update #2 -> output (h1 already unit-norm: ra=None)
        hm_bf = tmp.tile([128, C], bf16, tag="res_in")
        nc.vector.tensor_copy(hm_bf[:, 0:512], psp0)
        nc.vector.tensor_copy(hm_bf[:, 512:1024], psp1)
        out_f = tmp.tile([128, C], f32, tag="res_out", bufs=1)
        _residual_update(nc, tmp, out_f, h1_sb[:, r, :], None, hm_bf,
                         lrs_sb[:, 2 * C:3 * C], lrs_sb[:, 3 * C:4 * C])
        nc.sync.dma_start(io["out"][ts(r, 128), :], out_f)

    if debug_dump:
        nc.sync.dma_start(io["dbg_qT"], qT_sb)
        nc.sync.dma_start(io["dbg_kT"], kT_sb)
        nc.sync.dma_start(io["dbg_yT"], yT_sb)
        nc.sync.dma_start(io["dbg_v"], v_sb.rearrange("p a b -> p (a b)"))
        nc.sync.dma_start(io["dbg_h1"], h1_sb.rearrange("p a b -> p (a b)"))
        nc.sync.dma_start(io["dbg_x1q"], x1q_sb.rearrange("p a b -> p (a b)"))
        nc.sync.dma_start(io["dbg_xmq"], xmq_sb.rearrange("p a b -> p (a b)"))


_CACHE = {}


def _get_built(mock_cc=False, sim_safe=False, debug_dump=False):
    key = ("nc", mock_cc, sim_safe, debug_dump)
    if key in _CACHE:
        return _CACHE[key]
    nc = bacc.Bacc(get_trn_type() or "TRN2", target_bir_lowering=False,
                   debug=False, num_devices=NC_)
    io = {}
    for name, shape, dt in _INPUT_SPECS:
        io[name] = nc.dram_tensor(name, list(shape), dt, kind="ExternalInput").ap()
    io["out"] = nc.dram_tensor("out", [ROWS, C], f32, kind="ExternalOutput").ap()
    if debug_dump:
        for nm, shape, dt in [("dbg_qT", (128, NT), bf16), ("dbg_kT", (128, NT), bf16),
                              ("dbg_yT", (128, NT), bf16),
                              ("dbg_v", (128, 16 * 2 * (D + 1)), bf16),
                              ("dbg_h1", (128, 2 * C), f32),
                              ("dbg_x1q", (128, 8 * ROWS), fp8),
                              ("dbg_xmq", (128, NPAIR * ROWS), fp8)]:
            io[nm] = nc.dram_tensor(nm, list(shape), dt, kind="ExternalOutput").ap()
    with tile.TileContext(nc) as tc:
        _build_kernel(tc, io, mock_cc=mock_cc, sim_safe=sim_safe,
                      debug_dump=debug_dump)
    nc.compile()
    _CACHE[key] = nc
    return nc


def kernel(**inputs) -> np.ndarray:
    rope, perm128, masks = _host_tables()
    mlp_tabs = _mlp_tables(inputs)
    in_maps = [_core_inputs(g, inputs, rope, perm128, masks, mlp_tabs)
               for g in range(NC_)]
    nc = _get_built(
        sim_safe=bool(int(os.environ.get("KERNEL_SIM_SAFE", "0"))),
        debug_dump=bool(int(os.environ.get("KERNEL_DEBUG_DUMP", "0"))))
    trace = bool(int(os.environ.get("KERNEL_TRACE", "0")))
    res = run_bass_kernel_spmd(nc, in_maps, core_ids=list(range(NC_)), trace=trace)
    if trace and res.exec_time_ns is not None:
        print(f"HW exec time: {res.exec_time_ns} ns")
        _CACHE["exec_time_ns"] = res.exec_time_ns
        _CACHE["trace"] = res.instructions_and_trace
    _CACHE["results"] = res.results
    out = np.zeros((NT, C), np.float32)
    for g in range(NC_):
        og = res.results[g]["out"]
        out[g * 128:(g + 1) * 128] = og[0:128]
        out[T + g * 128:T + (g + 1) * 128] = og[128:256]
    return out.reshape(B, T, C).astype(np.float32)


if __name__ == "__main__":
    rng = np.random.default_rng(0)
    fake = {
        "h": rng.standard_normal((B, T, C), dtype=np.float32),
        "Wq": rng.standard_normal((C, C), dtype=np.float32) * 0.02,
        "Wk": rng.standard_normal((C, C), dtype=np.float32) * 0.02,
        "Wv": rng.standard_normal((C, C), dtype=np.float32) * 0.02,
        "Wo": rng.standard_normal((C, C), dtype=np.float32) * 0.02,
        "Wfc": rng.standard_normal((8 * C, C), dtype=np.float32) * 0.02,
        "Wproj": rng.standard_normal((C, 4 * C), dtype=np.float32) * 0.02,
        "sqk": BASE_SCALE * np.ones(C, np.float32),
        "suv": np.ones(8 * C, np.float32),
        "attn_alpha": BASE_SCALE * np.ones(C, np.float32),
        "mlp_alpha": BASE_SCALE * np.ones(C, np.float32),
        "thr_c": 1.6 * np.ones(H, np.float32),
        "stp": 10.0 * np.ones(H, np.float32),
    }
    out = kernel(**fake)
    print("out", out.shape, out.dtype, np.abs(out).mean())


# revision 16
# speedup vs baseline: 1.4499x; 1.0008x over previous
"""Self-contained Trainium2 Bass kernel for nn_Block_86028194939235 (sparse_attention).

nGPT-style block: gated-softmax causal attention + 8C MLP, B=2 T=1024 C=1024 H=16.

Sharding (8 cores, hardcoded):
  - attention: heads sharded (2 heads/core); partial att_c_proj output
    ReduceScatter(add) over token rows -> each core owns 256 rows.
  - residual/norm work: sharded over the 256 local rows.
  - MLP: token-sharded — every core runs the FULL 8C MLP for its own 256
    rows (fp8 Wfc/Wproj resident in SBUF, streamed in during attention).
    No AllGather, no second ReduceScatter.
  - final output: each core returns its 256 rows; host concatenates.

fp8(e4m3) for QKV projections and both MLP GEMMs (justnorm makes global
scales vanish); bf16 for attention core; f32 residual/norm math.
"""
import math
import os

import numpy as np
import ml_dtypes

import concourse.bass as bass
import concourse.bacc as bacc
import concourse.mybir as mybir
import concourse.tile as tile
from concourse.bass import ts, ds
from concourse.bass_utils import run_bass_kernel_spmd
from concourse.masks import make_identity
from concourse.tile import add_dep_helper as _add_dep
from concourse._compat import with_exitstack, get_trn_type
from contextlib import ExitStack

NC_ = 8
B, T, C, H, D = 2, 1024, 1024, 16, 64
NT = B * T                 # 2048 tokens
HPC = H // NC_             # 2 heads per core
GD = HPC * D               # 128
ROWS = NT // NC_           # 256 rows per core
FHID = 8 * C               # 8192 full mlp hidden (u+v)
NPAIR = 32                 # 32 (u,v) 128-row pairs
BASE_SCALE = 0.03125
SM_SCALE = math.sqrt(D)    # 8.0

# fp8 quantization scales (powers of two; all wash out through justnorm
# or are folded into suvu/suvv)
XS = 16.0      # h (attention input)
WS = 512.0     # Wq/Wk/Wv
FS = 1024.0    # Wfc
X1S = 128.0    # x1 (mlp input, unit-norm rows)
XMS = 8.0      # x_mlp (u*silu(v))
PS_ = 1024.0   # Wproj

bf16 = mybir.dt.bfloat16
f32 = mybir.dt.float32
fp8 = mybir.dt.float8e4
nbf = ml_dtypes.bfloat16
nf8 = ml_dtypes.float8_e4m3fn
AF = mybir.ActivationFunctionType
MUL = mybir.AluOpType.mult


def _to_bf(x):
    return np.ascontiguousarray(np.asarray(x, np.float32).astype(nbf))


def _to_f8(x, scale):
    a = np.asarray(x, np.float32) * scale
    return np.ascontiguousarray(np.clip(a, -240.0, 240.0).astype(nf8))


def _f32(x):
    return np.ascontiguousarray(np.asarray(x, np.float32))


# ---------------------------------------------------------------- host tables
def _sinusoidal_embeddings(n, d):
    pos = np.arange(n, dtype=np.float32)[:, None]
    div = np.exp(np.arange(0, d, 2, dtype=np.float32) * (-math.log(10000.0) / d))
    emb = np.zeros((n, d), dtype=np.float32)
    emb[:, 0::2] = np.sin(pos * div)
    emb[:, 1::2] = np.cos(pos * div)
    return emb


def _host_tables():
    emb = _sinusoidal_embeddings(T, D)
    R = np.zeros((D, T), dtype=np.float32)
    for i in range(D // 2):
        R[2 * i, :] = -emb[:, 32 + i]
        R[2 * i + 1, :] = emb[:, i]
    rope = np.tile(np.tile(R, (HPC, 1)), (1, B))          # (128, 2048)
    perm64 = np.arange(D).reshape(-1, 2)[:, ::-1].reshape(-1)
    perm128 = np.concatenate([perm64, perm64 + D])
    masks = np.zeros((4, 128, 512), dtype=np.float32)
    for m in range(4):
        masks[m] = (np.arange(512)[None, :] - 128 * m - np.arange(128)[:, None]) >= 0
    return rope, perm128, masks


def _mlp_tables(inp):
    """Full (unsharded) MLP weights, fp8, identical on every core."""
    Wfc = _f32(inp["Wfc"])                      # (8C, C)
    wfct = Wfc.T                                # (C, 8C): cols = hidden rows
    paired = np.empty((C, FHID), np.float32)
    for p in range(NPAIR):
        paired[:, p * 256:p * 256 + 128] = wfct[:, p * 128:(p + 1) * 128]
        paired[:, p * 256 + 128:p * 256 + 256] = \
            wfct[:, 4 * C + p * 128:4 * C + (p + 1) * 128]
    suv = _f32(inp["suv"]) * math.sqrt(C)
    suvu = suv[:4 * C].reshape(NPAIR, 128).T * (XMS / (X1S * FS))   # (128, 32)
    suvv = suv[4 * C:].reshape(NPAIR, 128).T * (1.0 / (X1S * FS))
    wprojt = _f32(inp["Wproj"]).T               # (4C, C): rows = hidden
    return (_to_f8(paired, FS), _f32(suvu), _f32(suvv), _to_f8(wprojt, PS_))


def _core_inputs(g, inp, rope, perm128, masks, mlp_tabs):
    h = _f32(inp["h"]).reshape(NT, C)
    gd = slice(g * GD, (g + 1) * GD)
    sqk_s = _f32(inp["sqk"])[gd] * (1.0 / BASE_SCALE)
    normind = np.zeros((2, GD), np.float32)
    normind[np.arange(GD) // D, np.arange(GD)] = sqk_s
    ones2 = np.zeros((GD, 2), np.float32)
    ones2[np.arange(GD), np.arange(GD) // D] = 1.0
    stp = _f32(inp["stp"])
    thr = _f32(inp["thr_c"])
    gatesc = np.zeros((128, 2), np.float32)
    gatebi = np.zeros((128, 2), np.float32)
    for hi in range(HPC):
        hh = HPC * g + hi
        gatesc[:, hi] = SM_SCALE * stp[hh]
        gatebi[:, hi] = -stp[hh] * thr[hh]
    lr1 = np.abs(_f32(inp["attn_alpha"]) * (0.05 / BASE_SCALE))
    lr2 = np.abs(_f32(inp["mlp_alpha"]) * (0.05 / BASE_SCALE))
    lrs = np.concatenate([1.0 - lr1, lr1, 1.0 - lr2, lr2])
    wfcq, suvu, suvv, wprojq = mlp_tabs
    return {
        "xtq": _to_f8(h.T, XS),
        "hloc": _f32(np.concatenate(
            [h[g * 128:(g + 1) * 128], h[T + g * 128:T + (g + 1) * 128]], 0)),
        "wqt": _to_f8(_f32(inp["Wq"])[gd][perm128].T, WS),
        "wkt": _to_f8(_f32(inp["Wk"])[gd][perm128].T, WS),
        "wvt": _to_f8(_f32(inp["Wv"])[gd].T, WS),
        "wot": _to_bf(_f32(inp["Wo"])[:, gd].T),
        "rope": _to_bf(rope),
        "normind": _f32(normind),
        "ones2": _to_bf(ones2),
        "masks": _to_f8(masks, 1.0),
        "gatesc": _f32(gatesc),
        "gatebi": _f32(gatebi),
        "wfcq": wfcq,
        "suvu": suvu,
        "suvv": suvv,
        "wprojq": wprojq,
        "lrs": np.ascontiguousarray(
            np.broadcast_to(lrs.reshape(1, 4 * C), (128, 4 * C)).astype(nbf)),
    }


_INPUT_SPECS = [
    ("xtq", (C, NT), fp8),
    ("hloc", (ROWS, C), f32),
    ("wqt", (C, GD), fp8),
    ("wkt", (C, GD), fp8),
    ("wvt", (C, GD), fp8),
    ("wot", (GD, C), bf16),
    ("rope", (GD, NT), bf16),
    ("normind", (2, GD), f32),
    ("ones2", (GD, 2), bf16),
    ("masks", (4, 128, 512), fp8),
    ("gatesc", (128, 2), f32),
    ("gatebi", (128, 2), f32),
    ("wfcq", (C, FHID), fp8),
    ("suvu", (128, NPAIR), f32),
    ("suvv", (128, NPAIR), f32),
    ("wprojq", (4 * C, C), fp8),
    ("lrs", (128, 4 * C), bf16),
]


# ---------------------------------------------------------------- device code
def _residual_update(nc, tmp, out_f32, a_sb, ra, upd_bf, lrm_ap, lrv_ap):
    """out = justnorm(lrm*(ra*a) + lrv*justnorm(upd)).

    a_sb: [128, C] f32 (A, pre-norm); ra: [128,1] f32 reciprocal-norm of a
          (pass None to treat a as already unit-norm).
    upd_bf: [128, C] bf16 (update branch, pre-norm; any global scale).
    """
    sq = tmp.tile([128, C], bf16, tag="res_sq")
    ssb = tmp.tile([128, 1], f32, tag="res_ss")
    nc.scalar.activation(sq, upd_bf, AF.Square, accum_out=ssb)
    srt = tmp.tile([128, 1], f32, tag="res_srt")
    nc.scalar.activation(srt, ssb, AF.Sqrt)
    rb = tmp.tile([128, 1], f32, tag="res_rb")
    nc.vector.reciprocal_approx_fast(rb, srt)

    t1 = tmp.tile([128, C], f32, tag="res_t1", bufs=1)
    if ra is None:
        nc.vector.tensor_mul(t1, a_sb, lrm_ap)
    else:
        nc.vector.scalar_tensor_tensor(
            t1, in0=a_sb, scalar=ra, in1=lrm_ap, op0=MUL, op1=MUL)
    t2 = tmp.tile([128, C], f32, tag="res_t2", bufs=1)
    nc.vector.scalar_tensor_tensor(
        t2, in0=upd_bf, scalar=rb, in1=lrv_ap, op0=MUL, op1=MUL)
    nc.vector.tensor_add(t1, t1, t2)
    sq2 = tmp.tile([128, C], bf16, tag="res_sq")
    ss2 = tmp.tile([128, 1], f32, tag="res_ss")
    nc.scalar.activation(sq2, t1, AF.Square, accum_out=ss2)
    srt2 = tmp.tile([128, 1], f32, tag="res_srt")
    nc.scalar.activation(srt2, ss2, AF.Sqrt)
    rs = tmp.tile([128, 1], f32, tag="res_rb")
    nc.vector.reciprocal_approx_fast(rs, srt2)
    nc.vector.tensor_scalar_mul(out_f32, t1, rs)


def _rnorm_of(nc, tmp, x_sb):
    """reciprocal L2 norm over free dim: [128, C] f32 -> [128,1] f32."""
    sq = tmp.tile([128, C], bf16, tag="res_sq")
    ssb = tmp.tile([128, 1], f32, tag="res_ss")
    nc.scalar.activation(sq, x_sb, AF.Square, accum_out=ssb)
    srt = tmp.tile([128, 1], f32, tag="res_srt")
    nc.scalar.activation(srt, ssb, AF.Sqrt)
    r = tmp.tile([128, 1], f32, tag="res_ra")
    nc.vector.reciprocal_approx_fast(r, srt)
    return r


@with_exitstack
def _build_kernel(ctx: ExitStack, tc: tile.TileContext, io: dict, mock_cc=False,
                  sim_safe=False, debug_dump=False):
    nc = tc.nc
    RG = [[i for i in range(NC_)]]

    # internal DRAM for the attention ReduceScatter
    hatt_part = nc.dram_tensor("hatt_part", [NT, C], bf16, kind="Internal").ap()
    hatt_rs = [nc.dram_tensor(f"hatt_rs{i}", [128, C], bf16, kind="Internal").ap()
               for i in range(2)]

    const = ctx.enter_context(tc.tile_pool(name="const", bufs=1))
    tmp = ctx.enter_context(tc.tile_pool(name="tmp", bufs=2))
    ps = ctx.enter_context(tc.tile_pool(name="ps", bufs=2, space="PSUM"))
    wstream = ctx.enter_context(tc.tile_pool(name="wstream", bufs=4))

    # ---- load constants / weights to SBUF (sync queue)
    wq_sb = const.tile([128, 8, GD], fp8, tag="wq")
    nc.sync.dma_start(wq_sb, io["wqt"].rearrange("(cc p) m -> p cc m", p=128))
    xt_sb = const.tile([128, 8, NT], fp8, tag="xmat")
    xt_view = io["xtq"].rearrange("(cc p) t -> p cc t", p=128)
    for ntc4 in range(4):
        for cc in range(8):
            nc.sync.dma_start(xt_sb[:, cc, ts(ntc4, 512)],
                              xt_view[:, cc, ts(ntc4, 512)])
    wk_sb = const.tile([128, 8, GD], fp8, tag="wk")
    nc.sync.dma_start(wk_sb, io["wkt"].rearrange("(cc p) m -> p cc m", p=128))
    wv_sb = const.tile([128, 8, GD], fp8, tag="wv")
    nc.sync.dma_start(wv_sb, io["wvt"].rearrange("(cc p) m -> p cc m", p=128))
    wo_sb = const.tile([128, C], bf16, tag="wo")
    nc.sync.dma_start(wo_sb, io["wot"])

    normind_sb = const.tile([2, GD], f32, tag="normind")
    nc.sync.dma_start(normind_sb, io["normind"])
    ones2_sb = const.tile([GD, 2], bf16, tag="ones2")
    nc.sync.dma_start(ones2_sb, io["ones2"])
    masks_sb = const.tile([128, 4, 512], fp8, tag="masks")
    nc.sync.dma_start(masks_sb, io["masks"].rearrange("m p q -> p m q"))
    gatesc_sb = const.tile([128, 2], f32, tag="gatesc")
    nc.sync.dma_start(gatesc_sb, io["gatesc"])
    gatebi_sb = const.tile([128, 2], f32, tag="gatebi")
    nc.sync.dma_start(gatebi_sb, io["gatebi"])
    suvu_sb = const.tile([128, NPAIR], f32, tag="suvu")
    nc.sync.dma_start(suvu_sb, io["suvu"])
    suvv_sb = const.tile([128, NPAIR], f32, tag="suvv")
    nc.sync.dma_start(suvv_sb, io["suvv"])
    lrs_sb = const.tile([128, 4 * C], bf16, tag="lrs")
    nc.sync.dma_start(lrs_sb, io["lrs"])
    ident_sb = const.tile([128, 128], bf16, tag="ident")
    make_identity(nc, ident_sb)
    ones164 = const.tile([34, D], f32, tag="ones164")
    nc.vector.memset(ones164, 1.0)

    # ---- full MLP weights (fp8, resident): stream on the ACT hwdge queue so
    # the sync queue keeps feeding attention. Issued first on that engine.
    wfcq_sb = const.tile([128, 8, FHID], fp8, tag="wfcq")
    wfcq_view = io["wfcq"].rearrange("(cc p) m -> p cc m", p=128)
    for cc in range(8):
        nc.scalar.dma_start(wfcq_sb[:, cc], wfcq_view[:, cc])

    qT_sb = const.tile([128, NT], bf16, tag="qT")
    kT_sb = const.tile([128, NT], bf16, tag="kT")
    v_sb = const.tile([128, 16, 2 * (D + 1)], bf16, tag="v")
    yT_sb = const.tile([128, NT], bf16, tag="yT")
    h1_sb = const.tile([128, 2, C], f32, tag="h1")
    x1q_sb = const.tile([128, 8, ROWS], fp8, tag="xmat")
    xmq_sb = const.tile([128, NPAIR, ROWS], fp8, tag="xmq")

    # ---- phase 1a: q/k projections with fused rope + head-norm + sqk scale
    for ntc in range(4):
        rope_sb = tmp.tile([128, 512], bf16, tag="rope", bufs=2,
                           name=f"rope_{ntc}")
        nc.sync.dma_start(rope_sb, io["rope"][:, ts(ntc, 512)])
        for w_sb, out_sb in ((wq_sb, qT_sb), (wk_sb, kT_sb)):
            psq = ps.tile([128, 512], f32, tag="mm", bufs=3)
            for cc in range(8):
                nc.tensor.matmul(psq, lhsT=w_sb[:, cc, :],
                                 rhs=xt_sb[:, cc, ts(ntc, 512)],
                                 start=cc == 0, stop=cc == 7)
            qrot = tmp.tile([128, 512], f32, tag="qrot", bufs=3)
            nc.vector.tensor_mul(qrot, psq, rope_sb)
            sq = tmp.tile([128, 512], bf16, tag="qsq")
            nc.vector.tensor_mul(sq, qrot, qrot)
            ssq = ps.tile([2, 512], f32, tag="y")
            nc.tensor.matmul(ssq, lhsT=ones2_sb, rhs=sq, start=True, stop=True)
            srtq = tmp.tile([2, 512], f32, tag="rpool")
            nc.scalar.activation(srtq, ssq, AF.Sqrt)
            rn = tmp.tile([2, 512], f32, tag="rpool")
            nc.vector.reciprocal_approx_fast(rn, srtq)
            bc = ps.tile([128, 512], f32, tag="aux")
            nc.tensor.matmul(bc, lhsT=normind_sb, rhs=rn, start=True, stop=True)
            nc.vector.tensor_mul(out_sb[:, ts(ntc, 512)], qrot, bc)

    # ---- phase 1b: v in [tok, head*(D+1)] layout with ones column
    nc.vector.memset(v_sb[:, :, D:D + 1], 1.0)
    nc.vector.memset(v_sb[:, :, 2 * D + 1:2 * D + 2], 1.0)
    for tci in range(16):
        psv = ps.tile([128, 128], f32, tag="mm", bufs=3)
        for cc in range(8):
            nc.tensor.matmul(psv, lhsT=xt_sb[:, cc, ts(tci, 128)],
                             rhs=wv_sb[:, cc, :], start=cc == 0, stop=cc == 7)
        # strided copy: both heads at once into the (D+1)-strided layout
        vout = v_sb[:, tci].rearrange("p (h e) -> p h e", h=2)[:, :, 0:D]
        vin = psv.rearrange("p (h d) -> p h d", h=2)
        nc.vector.tensor_copy(vout, vin)

    _ACT_CHAIN = [None, None]
    cc1 = None

    def _wo_and_rs(b):
        """partial att_c_proj for batch b's token rows, then row-split RS."""
        nonlocal cc1
        for tci in range(b * 8, b * 8 + 8):
            for ncc in range(2):
                pso = ps.tile([128, 512], f32, tag="mm", bufs=3,
                              name=f"pso_{tci}_{ncc}")
                nc.tensor.matmul(pso, lhsT=yT_sb[:, ts(tci, 128)],
                                 rhs=wo_sb[:, ts(ncc, 512)], start=True, stop=True)
                ha = tmp.tile([128, 512], bf16, tag="ha", name=f"ha_{tci}_{ncc}")
                nc.vector.tensor_copy(ha, pso)
                nc.sync.dma_start(hatt_part[ts(tci, 128), ts(ncc, 512)], ha)
        if mock_cc:
            cc1 = nc.sync.dma_start(hatt_rs[b][:], hatt_part[b * T:b * T + 128, :])
        else:
            cc1 = nc.gpsimd.collective_compute(
                "ReduceScatter", mybir.AluOpType.add, replica_groups=RG,
                ins=[hatt_part[b * T:(b + 1) * T, :]], outs=[hatt_rs[b][:]])

    # ---- phase 1c: attention per (batch, head)
    # Two passes over the (qc, kc) chunks per (b, head): pass 0 computes all
    # exp() chunks (Exp LUT loaded once), pass 1 recomputes scores on PE and
    # does Sigmoid + gating + PV (Sigmoid LUT loaded once) — avoids per-chunk
    # ACT table thrash (~1.3us per switch). Chunks on/below the causal
    # diagonal are width-trimmed to their live columns.
    def _attn_chunks(qc):
        n_kc = min(8, 4 * qc + 4)
        out = []
        for kc in range(n_kc):
            m = kc - 4 * qc
            off = max(0, 128 * m)   # first live column within the qc chunk
            out.append((kc, m, off, 512 - off))
        return out

    for b in range(B):
        for hi in range(HPC):
            dsl = ds(hi * D, D)
            y_aug = [ps.tile([D + 1, 512], f32, tag="y", name=f"y_{b}_{hi}_{qc2}")
                     for qc2 in range(2)]
            e_tiles = {}
            exp_insts = []
            for qc in range(2):
                for kc, m, off, w in _attn_chunks(qc):
                    s_ps = ps.tile([128, 512], f32, tag="mm", bufs=3)
                    nc.tensor.matmul(
                        s_ps[:, :w],
                        lhsT=kT_sb[dsl, ds(b * T + kc * 128, 128)],
                        rhs=qT_sb[dsl, ds(b * T + qc * 512 + off, w)],
                        start=True, stop=True)
                    e_sb = tmp.tile([128, 512], bf16, tag="e", bufs=13,
                                    name=f"e_{b}_{hi}_{qc}_{kc}")
                    ei = nc.scalar.activation(e_sb[:, :w], s_ps[:, :w], AF.Exp,
                                              scale=SM_SCALE)
                    exp_insts.append(ei)
                    e_tiles[(qc, kc)] = e_sb
            # ACT LUT grouping: first exp of this group after last sigmoid of
            # the previous group; first sigmoid after last exp of this group.
            if _ACT_CHAIN[0] is not None:
                _add_dep(exp_insts[0].ins, _ACT_CHAIN[0].ins,
                         reason="ACT table grouping: exp group after sigmoids")
            first_sig = [None]
            for qc in range(2):
                first = True
                chunks = _attn_chunks(qc)
                for kc, m, off, w in chunks:
                    s_ps = ps.tile([128, 512], f32, tag="mm", bufs=3)
                    nc.tensor.matmul(
                        s_ps[:, :w],
                        lhsT=kT_sb[dsl, ds(b * T + kc * 128, 128)],
                        rhs=qT_sb[dsl, ds(b * T + qc * 512 + off, w)],
                        start=True, stop=True)
                    g_sb = tmp.tile([128, 512], bf16, tag="g")
                    gi = nc.scalar.activation(g_sb[:, :w], s_ps[:, :w], AF.Sigmoid,
                                              scale=gatesc_sb[:, hi:hi + 1],
                                              bias=gatebi_sb[:, hi:hi + 1])
                    if first_sig[0] is None:
                        first_sig[0] = gi
                        _add_dep(gi.ins, exp_insts[-1].ins,
                                 reason="ACT table grouping: sigmoids after exps")
                    _ACT_CHAIN[0] = gi
                    p_sb = tmp.tile([128, 512], bf16, tag="p")
                    nc.vector.tensor_mul(p_sb[:, :w], e_tiles[(qc, kc)][:, :w],
                                         g_sb[:, :w])
                    if m >= 0:
                        nc.vector.tensor_mul(p_sb[:, :w], p_sb[:, :w],
                                             masks_sb[:, m, off:512])
                    nc.tensor.matmul(
                        y_aug[qc][:, off:512],
                        lhsT=v_sb[:, b * 8 + kc, ds(hi * (D + 1), D + 1)],
                        rhs=p_sb[:, :w],
                        start=first, stop=kc == chunks[-1][0])
                    first = False
            # renormalize: yT = y[:D] / y[D]. Both qc denominators go into
            # one [2,512] tile so the custom-DVE reciprocal runs on a
            # multi-partition shape (single-partition [1,512] miscomputes on
            # HW).
            den2 = tmp.tile([34, 512], f32, tag="rpool")
            nc.scalar.copy(den2[0:1, :], y_aug[0][D:D + 1, :])
            nc.scalar.copy(den2[32:33, :], y_aug[1][D:D + 1, :])
            rcp2 = tmp.tile([34, 512], f32, tag="rpool")
            nc.vector.reciprocal_approx_fast(rcp2, den2)
            for qc in range(2):
                rb = ps.tile([D, 512], f32, tag="aux")
                nc.tensor.matmul(rb, lhsT=ones164[ds(32 * qc, 1), :],
                                 rhs=rcp2[ds(32 * qc, 1), :],
                                 start=True, stop=True)
                rb_sb = tmp.tile([D, 512], f32, tag="rpool")
                nc.vector.tensor_copy(rb_sb, rb)
                nc.vector.tensor_mul(
                    yT_sb[dsl, ds(b * T + qc * 512, 512)], y_aug[qc][:D, :], rb_sb)
        _wo_and_rs(b)

    # ---- residual update #1 (local 256 rows) + transpose to fp8 x1^T
    for r in range(2):
        ha_bf = tmp.tile([128, C], bf16, tag="res_in")
        nc.sync.dma_start(ha_bf, hatt_rs[r][:])
        hloc_sb = tmp.tile([128, C], f32, tag="res_hloc", bufs=1)
        nc.sync.dma_start(hloc_sb, io["hloc"][ts(r, 128), :])
        ra = _rnorm_of(nc, tmp, hloc_sb)
        _residual_update(nc, tmp, h1_sb[:, r, :], hloc_sb, ra, ha_bf,
                         lrs_sb[:, 0:C], lrs_sb[:, C:2 * C])
        # h1 scaled by X1S into bf16, then PE-transpose into fp8 x1^T tiles
        h1b = tmp.tile([128, C], bf16, tag="res_bf")
        nc.scalar.activation(h1b, h1_sb[:, r, :], AF.Copy, scale=X1S)
        for cc in range(8):
            tps = ps.tile([128, 128], bf16, tag="aux")
            nc.tensor.transpose(tps, h1b[:, ts(cc, 128)], ident_sb)
            nc.vector.tensor_copy(x1q_sb[:, cc, ts(r, 128)], tps)

    # ---- stream Wproj (fp8) on the sync queue in 8 half-MB chunks
    # (4 hc-slices each). Wave 0 of c_proj reads chunks 0..7 ascending; wave 1
    # reads 7..0 descending, so chunks 4..7 are still resident and only 0..3
    # are re-fetched (issued between the waves).
    wprojq_view = io["wprojq"].rearrange("(hc p) m -> p hc m", p=128)
    wp_cur = {}

    def _load_wp(wc, gen):
        wp = wstream.tile([128, 4, C], fp8, tag="wproj", name=f"wp_{wc}_{gen}")
        nc.sync.dma_start(wp, wprojq_view[:, ts(wc, 4)])
        wp_cur[wc] = wp

    for wc in range(8):
        _load_wp(wc, 0)

    # ---- MLP (token-sharded, full hidden on this core): fc + silu
    for pr in range(NPAIR):
        psu = ps.tile([128, ROWS], f32, tag="mm", bufs=3)
        psv2 = ps.tile([128, ROWS], f32, tag="mm", bufs=3)
        for cc in range(8):
            nc.tensor.matmul(psu, lhsT=wfcq_sb[:, cc, ds(pr * 256, 128)],
                             rhs=x1q_sb[:, cc, :], start=cc == 0, stop=cc == 7)
        for cc in range(8):
            nc.tensor.matmul(psv2, lhsT=wfcq_sb[:, cc, ds(pr * 256 + 128, 128)],
                             rhs=x1q_sb[:, cc, :], start=cc == 0, stop=cc == 7)
        sv = tmp.tile([128, ROWS], bf16, tag="silu")
        if sim_safe:
            sg = tmp.tile([128, ROWS], bf16, tag="sg")
            nc.scalar.activation(sg, psv2, AF.Sigmoid,
                                 scale=suvv_sb[:, pr:pr + 1])
            nc.vector.scalar_tensor_tensor(
                sv, in0=psv2, scalar=suvv_sb[:, pr:pr + 1],
                in1=sg, op0=MUL, op1=MUL)
        else:
            nc.scalar.activation(sv, psv2, AF.Silu,
                                 scale=suvv_sb[:, pr:pr + 1])
        nc.vector.scalar_tensor_tensor(
            xmq_sb[:, pr, :], in0=psu, scalar=suvu_sb[:, pr:pr + 1],
            in1=sv, op0=MUL, op1=MUL)

    # ---- c_proj: h_mlp[tok, C] = x_mlp^T @ Wproj^T, contraction over hidden.
    # Two sequential token waves (2 PSUM banks each, reusing the "y" slots);
    # wave r's residual update overlaps wave r+1's matmuls.
    for r in range(2):
        if r == 1:
            for wc in (3, 2, 1, 0):
                _load_wp(wc, 1)
        psp0 = ps.tile([128, 512], f32, tag="y", name=f"psp0_{r}")
        psp1 = ps.tile([128, 512], f32, tag="y", name=f"psp1_{r}")
        hcs = range(NPAIR) if r == 0 else range(NPAIR - 1, -1, -1)
        for i, hc in enumerate(hcs):
            wp = wp_cur[hc // 4]
            nc.tensor.matmul(psp0, lhsT=xmq_sb[:, hc, ts(r, 128)],
                             rhs=wp[:, hc % 4, 0:512],
                             start=i == 0, stop=i == NPAIR - 1)
            nc.tensor.matmul(psp1, lhsT=xmq_sb[:, hc, ts(r, 128)],
                             rhs=wp[:, hc % 4, 512:1024],
                             start=i == 0, stop=i == NPAIR - 1)
        # ---- residual update #2 -> output (h1 already unit-norm: ra=None)
        hm_bf = tmp.tile([128, C], bf16, tag="res_in")
        nc.vector.tensor_copy(hm_bf[:, 0:512], psp0)
        nc.vector.tensor_copy(hm_bf[:, 512:1024], psp1)
        out_f = tmp.tile([128, C], f32, tag="res_out", bufs=1)
        _residual_update(nc, tmp, out_f, h1_sb[:, r, :], None, hm_bf,
                         lrs_sb[:, 2 * C:3 * C], lrs_sb[:, 3 * C:4 * C])
        nc.sync.dma_start(io["out"][ts(r, 128), :], out_f)

    if debug_dump:
        nc.sync.dma_start(io["dbg_qT"], qT_sb)
        nc.sync.dma_start(io["dbg_kT"], kT_sb)
        nc.sync.dma_start(io["dbg_yT"], yT_sb)
        nc.sync.dma_start(io["dbg_v"], v_sb.rearrange("p a b -> p (a b)"))
        nc.sync.dma_start(io["dbg_h1"], h1_sb.rearrange("p a b -> p (a b)"))
        nc.sync.dma_start(io["dbg_x1q"], x1q_sb.rearrange("p a b -> p (a b)"))
        nc.sync.dma_start(io["dbg_xmq"], xmq_sb.rearrange("p a b -> p (a b)"))


_CACHE = {}


def _get_built(mock_cc=False, sim_safe=False, debug_dump=False):
    key = ("nc", mock_cc, sim_safe, debug_dump)
    if key in _CACHE:
        return _CACHE[key]
    nc = bacc.Bacc(get_trn_type() or "TRN2", target_bir_lowering=False,
                   debug=False, num_devices=NC_)
    io = {}
    for name, shape, dt in _INPUT_SPECS:
        io[name] = nc.dram_tensor(name, list(shape), dt, kind="ExternalInput").ap()
    io["out"] = nc.dram_tensor("out", [ROWS, C], f32, kind="ExternalOutput").ap()
    if debug_dump:
        for nm, shape, dt in [("dbg_qT", (128, NT), bf16), ("dbg_kT", (128, NT), bf16),
                              ("dbg_yT", (128, NT), bf16),
                              ("dbg_v", (128, 16 * 2 * (D + 1)), bf16),
                              ("dbg_h1", (128, 2 * C), f32),
                              ("dbg_x1q", (128, 8 * ROWS), fp8),
                              ("dbg_xmq", (128, NPAIR * ROWS), fp8)]:
            io[nm] = nc.dram_tensor(nm, list(shape), dt, kind="ExternalOutput").ap()
    with tile.TileContext(nc) as tc:
        _build_kernel(tc, io, mock_cc=mock_cc, sim_safe=sim_safe,
                      debug_dump=debug_dump)
    nc.compile()
    _CACHE[key] = nc
    return nc


def kernel(**inputs) -> np.ndarray:
    rope, perm128, masks = _host_tables()
    mlp_tabs = _mlp_tables(inputs)
    in_maps = [_core_inputs(g, inputs, rope, perm128, masks, mlp_tabs)
               for g in range(NC_)]
    nc = _get_built(
        sim_safe=bool(int(os.environ.get("KERNEL_SIM_SAFE", "0"))),
        debug_dump=bool(int(os.environ.get("KERNEL_DEBUG_DUMP", "0"))))
    trace = bool(int(os.environ.get("KERNEL_TRACE", "0")))
    res = run_bass_kernel_spmd(nc, in_maps, core_ids=list(range(NC_)), trace=trace)
    if trace and res.exec_time_ns is not None:
        print(f"HW exec time: {res.exec_time_ns} ns")
        _CACHE["exec_time_ns"] = res.exec_time_ns
        _CACHE["trace"] = res.instructions_and_trace
    _CACHE["results"] = res.results
    out = np.zeros((NT, C), np.float32)
    for g in range(NC_):
        og = res.results[g]["out"]
        out[g * 128:(g + 1) * 128] = og[0:128]
        out[T + g * 128:T + (g + 1) * 128] = og[128:256]
    return out.reshape(B, T, C).astype(np.float32)


if __name__ == "__main__":
    rng = np.random.default_rng(0)
    fake = {
        "h": rng.standard_normal((B, T, C), dtype=np.float32),
        "Wq": rng.standard_normal((C, C), dtype=np.float32) * 0.02,
        "Wk": rng.standard_normal((C, C), dtype=np.float32) * 0.02,
        "Wv": rng.standard_normal((C, C), dtype=np.float32) * 0.02,
        "Wo": rng.standard_normal((C, C), dtype=np.float32) * 0.02,
        "Wfc": rng.standard_normal((8 * C, C), dtype=np.float32) * 0.02,
        "Wproj": rng.standard_normal((C, 4 * C), dtype=np.float32) * 0.02,
        "sqk": BASE_SCALE * np.ones(C, np.float32),
        "suv": np.ones(8 * C, np.float32),
        "attn_alpha": BASE_SCALE * np.ones(C, np.float32),
        "mlp_alpha": BASE_SCALE * np.ones(C, np.float32),
        "thr_c": 1.6 * np.ones(H, np.float32),
        "stp": 10.0 * np.ones(H, np.float32),
    }
    out = kernel(**fake)
    print("out", out.shape, out.dtype, np.abs(out).mean())
